# revision 10
# baseline (speedup 1.0000x reference)
"""Causal self-attention (GQA + RMS-norm + RoPE) Trainium2 Bass kernel.

Sharding over 8 NeuronCores: 2-way data parallel (batch) x 4-way head
parallel (one GQA group of 4 q-heads + 1 kv-head per core).  Each core
computes q/k/v projections for its group, flash-style causal attention
(scores kept transposed [k, q] so softmax sums ride the tensor engine),
and a partial output projection.  Host sums the 4 per-group partials per
batch.

Engine-balance notes (from NTFF traces):
 - All activations stay in one ACT table set (natural_log_exp_and_others):
   RMS-norm rsqrt and the softmax 1/d are exp(-ln(x)) instead of
   sqrt/reciprocal.  DVE's iterative-divide reciprocal costs ~3.3us per
   [1,512] row; ln+exp on ACT is ~1.4us and avoids table switching.
 - RoPE stages are bf16 so the DVE tensor-tensor ops run in 2x mode.
 - x loads are column-major (j-block at a time) so the first projection
   chains start ~12us into the kernel instead of after the full 8MB load.
 - Off-diagonal attention tiles run the PV and denominator matmuls in
   fp8 DoubleRow pairs (2 k-tiles per instruction); exp emits P in fp8
   with bias -5 (cancels in y/d).  Verified <=4.2e-3 rel err vs 2e-2 gate.
 - Output partials are bf16 (halves the store and the host gather).
"""

import numpy as np
import ml_dtypes

import concourse.bacc as bacc
import concourse.mybir as mybir
from concourse.tile import TileContext
from concourse.bass_utils import run_bass_kernel_spmd

BF16 = mybir.dt.bfloat16
F32 = mybir.dt.float32
F32R = mybir.dt.float32r
FP8 = mybir.dt.float8e4
AF = mybir.ActivationFunctionType
DR = mybir.MatmulPerfMode.DoubleRow
bf = ml_dtypes.bfloat16
f8 = ml_dtypes.float8_e4m3

B, S, D = 2, 2048, 2048
H, HKV, HD = 16, 4, 128
RQ = H // HKV            # q heads per kv group (4)
NCORES = 8
NDT = D // 128           # 16 contraction tiles
NST = S // 512           # 4 query/sequence 512-tiles
NKT = S // 128           # 16 key 128-tiles
EPS = float(np.finfo(np.float32).eps)
EXPB = -5.0              # softmax exp bias: P = e^(s-5); cancels in y/d

_PROG_CACHE = {}


def _build_program(n_timing_iters=1, phases="full"):
    nc = bacc.Bacc("TRN2", debug=False, enable_asserts=False, num_devices=NCORES)

    xT_d = nc.dram_tensor("xT", [128, NDT, S], BF16, kind="ExternalInput")
    wqT_d = nc.dram_tensor("wqT", [128, NDT, RQ * HD], BF16, kind="ExternalInput")
    wkT_d = nc.dram_tensor("wkT", [128, NDT, HD], BF16, kind="ExternalInput")
    wvT_d = nc.dram_tensor("wvT", [128, NDT, HD], BF16, kind="ExternalInput")
    wpT_d = nc.dram_tensor("wpT", [NDT, 128, RQ * 128], BF16, kind="ExternalInput")
    cosF_d = nc.dram_tensor("cosF", [128, S], BF16, kind="ExternalInput")
    sinF_d = nc.dram_tensor("sinF", [128, S], BF16, kind="ExternalInput")
    cfs_d = nc.dram_tensor("cfs", [1, 641], F32R, kind="ExternalInput")
    onescol_f_d = nc.dram_tensor("onescol_f", [128, 1], F32R, kind="ExternalInput")
    expb_d = nc.dram_tensor("expb", [128, 1], F32R, kind="ExternalInput")
    onescol_b_d = nc.dram_tensor("onescol_b", [128, 1], BF16, kind="ExternalInput")
    ones8_d = nc.dram_tensor("ones8", [128, 32], FP8, kind="ExternalInput")
    idtr_d = nc.dram_tensor("idtr", [128, 256], BF16, kind="ExternalInput")
    outT_d = nc.dram_tensor("outT", [NDT, 128, S], BF16, kind="ExternalOutput")

    with TileContext(nc) as tc:
        with tc.tile_pool(name="res", bufs=1) as res, \
             tc.tile_pool(name="work", bufs=2) as wk, \
             tc.tile_pool(name="pwork", bufs=2, space="PSUM") as pw:

            # ---- resident tiles (allocated once) ----
            xT = res.tile([128, NDT, S], BF16)             # [d-part, dt, s]
            wqT = res.tile([128, NDT, RQ * HD], BF16)
            wkT = res.tile([128, NDT, HD], BF16)
            wvT = res.tile([128, NDT, HD], BF16)
            cosF = res.tile([128, S], BF16)
            sinF = res.tile([128, S], BF16)
            cfs = res.tile([1, 641], F32R)
            onescol_f = res.tile([128, 1], F32R)
            expb = res.tile([128, 1], F32R)
            onescol_b = res.tile([128, 1], BF16)
            ones8 = res.tile([128, 2, 16], FP8)
            idtr = res.tile([128, 256], BF16)
            kT = res.tile([128, S], BF16)
            qT = [res.tile([128, S], BF16, name=f"qT{h}", tag=f"qT{h}")
                  for h in range(RQ)]
            yT = [res.tile([128, S], BF16, name=f"yT{h}", tag=f"yT{h}")
                  for h in range(RQ)]
            vTst = res.tile([128, S], BF16)                # v^T staging
            V_all = res.tile([128, S], BF16)               # v natural, kt-major
            V8 = res.tile([128, NKT, 128], FP8)            # fp8 copy of V_all

            eps_ap = cfs[0:1, 0:1].bitcast(F32)
            expb_ap = expb[:, 0:1].bitcast(F32)
            ident = idtr[:, 0:128]
            triu = idtr[:, 128:256]

            def body(_iv=None):
                # ---- load residents ----
                # small constants + k/v weights first (first chains), then x
                # column-major per j-block so chain j can start as soon as
                # its block lands, then q weights (needed ~30us in).
                nc.sync.dma_start(wkT[:], wkT_d[:])
                nc.sync.dma_start(wvT[:], wvT_d[:])
                nc.sync.dma_start(cfs[:], cfs_d[:])
                nc.sync.dma_start(onescol_f[:], onescol_f_d[:])
                nc.sync.dma_start(expb[:], expb_d[:])
                nc.sync.dma_start(onescol_b[:], onescol_b_d[:])
                nc.sync.dma_start(ones8[:], ones8_d[:])
                nc.sync.dma_start(idtr[:], idtr_d[:])
                nc.sync.dma_start(cosF[:], cosF_d[:])
                nc.sync.dma_start(sinF[:], sinF_d[:])
                for j in range(NST):
                    sl = slice(512 * j, 512 * j + 512)
                    nc.sync.dma_start(xT[:, 0:8, sl], xT_d[:, 0:8, sl])
                    nc.sync.dma_start(xT[:, 8:16, sl], xT_d[:, 8:16, sl])
                nc.sync.dma_start(wqT[:, 0:8, :], wqT_d[:, 0:8, :])
                nc.sync.dma_start(wqT[:, 8:16, :], wqT_d[:, 8:16, :])

                def proj_accum(wt_all, col_off, j, tag="big", tbufs=3):
                    """psum [128,512] = sum_d W[d].T @ xT[d, s-slice]"""
                    acc = pw.tile([128, 512], F32, name="acc", tag=tag,
                                  bufs=tbufs)
                    for dt in range(NDT):
                        nc.tensor.matmul(acc[:],
                                         wt_all[:, dt, col_off:col_off + 128],
                                         xT[:, dt, 512 * j:512 * j + 512],
                                         start=(dt == 0), stop=(dt == NDT - 1))
                    return acc

                def norm_rope_chain(acc, lg_ap, dest, j):
                    """RMS-norm + RoPE + scale; writes dest[:, 512j:+512] bf16.

                    rsqrt (and the q gain) ride ACT as exp(-0.5*ln(ms)+lg) so
                    everything stays in one activation table set.  Stages are
                    bf16 so the DVE rope ops run in 2x mode."""
                    stage = wk.tile([128, 512], BF16, name="stage", tag="stage", bufs=3)
                    nc.scalar.copy(stage[:], acc[:])
                    swap = wk.tile([128, 512], BF16, name="swap", tag="swap", bufs=3)
                    nc.sync.dma_start(swap[0:64, :], stage[64:128, :])
                    nc.sync.dma_start(swap[64:128, :], stage[0:64, :])
                    sq = wk.tile([128, 512], F32R, name="sq", tag="sq", bufs=2)
                    nc.scalar.square(sq[:], stage[:])
                    ms = pw.tile([1, 512], F32, name="ms", tag="bcast", bufs=1)
                    nc.tensor.matmul(ms[:], onescol_f[:],
                                     sq[:], start=True, stop=True)
                    lnms = wk.tile([1, 512], F32, name="lnms", tag="lnms", bufs=2)
                    nc.scalar.activation(lnms[:], ms[:], AF.Ln,
                                         bias=eps_ap, scale=1.0 / HD)
                    rg = wk.tile([1, 512], BF16, name="rg", tag="rg", bufs=2)
                    nc.scalar.activation(rg[:], lnms[:], AF.Exp,
                                         bias=(lg_ap if lg_ap is not None
                                               else 0.0), scale=-0.5)
                    Rb = wk.tile([128, 512], BF16, name="Rb", tag="Rb", bufs=2)
                    nc.gpsimd.partition_broadcast(Rb[:], rg[0:1, :])
                    sl = slice(512 * j, 512 * j + 512)
                    nc.vector.tensor_mul(stage[:], stage[:], cosF[:, sl])
                    nc.vector.tensor_mul(swap[:], swap[:], sinF[:, sl])
                    nc.vector.tensor_add(stage[:], stage[:], swap[:])
                    nc.vector.tensor_mul(dest[:, sl], stage[:], Rb[:])

                # ---- projections: mm-chains pipelined one ahead of stats ----
                proj_jobs = []   # (kind, h, j) interleaved to match x arrival
                for j in range(NST):
                    proj_jobs.append(("k", 0, j))
                    proj_jobs.append(("v", 0, j))

                def emit_proj_tail(kind, h, j, acc):
                    if kind == "k":
                        norm_rope_chain(acc, None, kT, j)
                    elif kind == "q":
                        lg_h = cfs[0:1, 133 + h:134 + h].bitcast(F32)
                        norm_rope_chain(acc, lg_h, qT[h], j)
                    else:
                        nc.scalar.copy(vTst[:, 512 * j:512 * j + 512], acc[:])

                def run_proj_jobs(jobs, pending, tags=None, lag=1):
                    for idx, (kind, h, j) in enumerate(jobs):
                        tag, tbufs = ("big", 3) if tags is None else tags[idx]
                        if kind == "k":
                            acc = proj_accum(wkT, 0, j, tag, tbufs)
                        elif kind == "v":
                            acc = proj_accum(wvT, 0, j, tag, tbufs)
                        else:
                            acc = proj_accum(wqT, 128 * h, j, tag, tbufs)
                        pending.append((kind, h, j, acc))
                        if len(pending) > lag:
                            emit_proj_tail(*pending.pop(0))
                    return pending

                # first wave: 7-deep psum concurrency so the x-load DMA
                # wavefront keeps PE fed (every arriving x tile unlocks mms)
                kv_tags = [("big", 3), ("big", 3), ("big", 3), ("acc", 2),
                           ("acc", 2), ("small", 2), ("small", 2), ("big", 3)]
                pending = run_proj_jobs(proj_jobs, [], tags=kv_tags, lag=6)

                # ---- v^T -> V transposes (PE); V kept in bf16 and fp8 ----
                while pending:
                    emit_proj_tail(*pending.pop(0))
                for kt in range(NKT):
                    tp = pw.tile([128, 128], BF16, name="tp", tag="acc", bufs=2)
                    nc.tensor.transpose(tp[:], vTst[:, 128 * kt:128 * kt + 128],
                                        ident)
                    nc.scalar.copy(V_all[:, 128 * kt:128 * kt + 128], tp[:])
                    nc.scalar.copy(V8[:, kt, :], tp[:])

                if phases == "kv":
                    return

                def attention_block(h, j):
                    """Causal attention for queries [512j, 512j+512), head h.

                    Off-diagonal k-tiles (kt < 4j) are consumed as fp8
                    DoubleRow pairs (P in fp8 from exp, V from V8): one PV and
                    one denominator matmul per TWO k-tiles.  Diagonal tiles
                    stay bf16 with column trimming + triu mask.  Consumers
                    trail the S-matmul/exp pipeline so PE never waits on ACT."""
                    nkt = 4 * j + 4
                    npair = (4 * j) // 2
                    acc_y = pw.tile([128, 512], F32, name="acc_y", tag="acc",
                                    bufs=2)
                    acc_d = pw.tile([1, 512], F32, name="acc_d", tag="small",
                                    bufs=2)
                    ncons = npair + 4
                    lagged = []

                    def consume(ci, kind, P, c0):
                        first, last = ci == 0, ci == ncons - 1
                        if kind == "pair":
                            nc.tensor.matmul(acc_d[:], ones8[:, 0:2, 0:1],
                                             P[:, 0:2, :],
                                             start=first, stop=last,
                                             perf_mode=DR,
                                             skip_group_check=True)
                            nc.tensor.matmul(acc_y[:], V8[:, c0:c0 + 2, :],
                                             P[:, 0:2, :],
                                             start=first, stop=last,
                                             perf_mode=DR,
                                             skip_group_check=True)
                        else:
                            kt = c0
                            cc = 128 * (kt - 4 * j)
                            nc.tensor.matmul(acc_d[0:1, cc:512], onescol_b[:],
                                             P[:, cc:512],
                                             start=first, stop=last,
                                             skip_group_check=True)
                            nc.tensor.matmul(acc_y[:, cc:512],
                                             V_all[:, 128 * kt:128 * kt + 128],
                                             P[:, cc:512],
                                             start=first, stop=last,
                                             skip_group_check=True)

                    ci = 0
                    for pi in range(npair):
                        P8 = wk.tile([128, 2, 512], FP8, name="P8", tag="P8",
                                     bufs=4)
                        for u in (0, 1):
                            kt = 2 * pi + u
                            ps = pw.tile([128, 512], F32, name="ps", tag="big",
                                         bufs=3)
                            nc.tensor.matmul(
                                ps[:], kT[:, 128 * kt:128 * kt + 128],
                                qT[h][:, 512 * j:512 * j + 512],
                                start=True, stop=True)
                            nc.scalar.activation(P8[:, u, :], ps[:], AF.Exp,
                                                 bias=expb_ap)
                        lagged.append((ci, "pair", P8, 2 * pi))
                        ci += 1
                        if len(lagged) > 2:
                            consume(*lagged.pop(0))
                    for kt in range(4 * j, nkt):
                        cc = 128 * (kt - 4 * j)
                        ps = pw.tile([128, 512], F32, name="ps", tag="big",
                                     bufs=3)
                        nc.tensor.matmul(
                            ps[:, cc:512],
                            kT[:, 128 * kt:128 * kt + 128],
                            qT[h][:, 512 * j + cc:512 * j + 512],
                            start=True, stop=True)
                        P = wk.tile([128, 512], BF16, name="P", tag="P", bufs=4)
                        nc.scalar.activation(P[:, cc:512], ps[:, cc:512],
                                             AF.Exp, bias=expb_ap)
                        nc.vector.tensor_mul(P[:, cc:cc + 128],
                                             P[:, cc:cc + 128], triu)
                        lagged.append((ci, "diag", P, kt))
                        ci += 1
                        if len(lagged) > 2:
                            consume(*lagged.pop(0))
                    while lagged:
                        consume(*lagged.pop(0))
                    # 1/d = exp(-ln(d)) on ACT (stays in the same table set)
                    lnd = wk.tile([1, 512], F32, name="lnd", tag="lnms", bufs=2)
                    nc.scalar.activation(lnd[:], acc_d[:], AF.Ln)
                    rdv = wk.tile([1, 512], BF16, name="rdv", tag="rg", bufs=2)
                    nc.scalar.activation(rdv[:], lnd[:], AF.Exp, scale=-1.0)
                    Rd = wk.tile([128, 512], BF16, name="Rd", tag="Rb", bufs=2)
                    nc.gpsimd.partition_broadcast(Rd[:], rdv[0:1, :])
                    nc.vector.tensor_mul(yT[h][:, 512 * j:512 * j + 512],
                                         acc_y[:], Rd[:])

                # ---- per q-head: q(h+1) projections emitted ahead of
                # attention(h) so PE crosses head boundaries without gaps ----
                pending = run_proj_jobs([("q", 0, j) for j in range(NST)],
                                        pending)
                for h in range(RQ):
                    if h + 1 < RQ:
                        pending = run_proj_jobs(
                            [("q", h + 1, j) for j in range(NST)], pending)
                    while pending:
                        emit_proj_tail(*pending.pop(0))
                    if phases == "kvq":
                        continue
                    for j in range(NST):
                        attention_block(h, j)

                # ---- output projection (transposed: out^T[D, s]) ----
                if phases in ("kv", "kvq", "noout"):
                    return
                ptags = ["big", "acc", "bcast", "small"]
                pbufs = {"big": 3, "acc": 2, "bcast": 1, "small": 2}
                for dt in range(NDT):
                    wp = wk.tile([128, RQ * 128], BF16, name="wp", tag="wp",
                                 bufs=3)
                    nc.sync.dma_start(wp[:], wpT_d[dt])
                    osb = wk.tile([128, S], BF16, name="osb", tag="osb")
                    for sjj in range(NST):
                        po = pw.tile([128, 512], F32, name=f"po{sjj}",
                                     tag=ptags[sjj], bufs=pbufs[ptags[sjj]])
                        for h in range(RQ):
                            nc.tensor.matmul(
                                po[:], wp[:, 128 * h:128 * h + 128],
                                yT[h][:, 512 * sjj:512 * sjj + 512],
                                start=(h == 0), stop=(h == RQ - 1))
                        # psum->sbuf copies split across ACT and DVE
                        if sjj % 2 == 0:
                            nc.scalar.copy(osb[:, 512 * sjj:512 * sjj + 512],
                                           po[:])
                        else:
                            nc.vector.tensor_copy(
                                osb[:, 512 * sjj:512 * sjj + 512], po[:])
                    nc.sync.dma_start(outT_d[dt], osb[:])

            if n_timing_iters > 1:
                with tc.For_i(0, n_timing_iters, 1):
                    body()
            else:
                body()

    nc.compile()
    return nc


def _get_program(n_timing_iters=1, phases="full"):
    key = (n_timing_iters, phases)
    if key not in _PROG_CACHE:
        _PROG_CACHE[key] = _build_program(n_timing_iters, phases)
    return _PROG_CACHE[key]


def _host_inputs(x, Wq, Wk, Wv, Wproj, q_gain):
    """Build the 8 per-core input maps (host-side layout prep)."""
    inv = 1.0 / (10000.0 ** (np.arange(0, HD, 2, dtype=np.float64) / HD))
    t = np.arange(S, dtype=np.float64)
    fr = np.outer(t, inv).astype(np.float32)          # [S, 64]
    cos = np.cos(fr).astype(np.float32)
    sin = np.sin(fr).astype(np.float32)
    cosF = np.concatenate([cos.T, cos.T], 0).astype(bf)          # [128, S]
    sinF = np.concatenate([sin.T, -sin.T], 0).astype(bf)

    onescol_f = np.ones((128, 1), np.float32)
    onescol_b = np.ones((128, 1), bf)
    ones8 = np.ones((128, 32), f8)
    ident = np.eye(128, dtype=np.float32)
    triu = (np.arange(128)[None, :] >= np.arange(128)[:, None]).astype(np.float32)
    idtr = np.concatenate([ident, triu], 1).astype(bf)

    # fold gain sign into Wq rows; |gain| rides the rsqrt exp as a ln-bias
    gsign = np.where(q_gain < 0, -1.0, 1.0).astype(np.float32)
    Wq = Wq * np.repeat(gsign, HD)[:, None]
    glog = np.log(np.maximum(np.abs(q_gain.astype(np.float64)), 1e-300)
                  / np.sqrt(HD))
    glog = np.maximum(glog, -80.0).astype(np.float32)

    # [128, NDT, S]: xT[p, dt, s] = x[b][s, 128*dt+p]
    xT = [np.ascontiguousarray(
        x[b].T.reshape(NDT, 128, S).transpose(1, 0, 2)).astype(bf)
        for b in range(B)]

    in_maps = []
    for c in range(NCORES):
        b, g = c // HKV, c % HKV
        wq = np.ascontiguousarray(Wq[512 * g:512 * (g + 1)].T)   # [D, 512]
        wk_ = np.ascontiguousarray(Wk[128 * g:128 * (g + 1)].T)  # [D, 128]
        wv = np.ascontiguousarray(Wv[128 * g:128 * (g + 1)].T)
        wpT = np.ascontiguousarray(Wproj[:, 512 * g:512 * (g + 1)].T)  # [512, 2048]
        # [dt][c-part 128, (h, m) 512]: wpT2[dt, c, 128h+m] = Wp[128dt+m, 512g+128h+c]
        wpT = np.ascontiguousarray(
            wpT.reshape(RQ, 128, NDT, 128).transpose(2, 1, 0, 3).reshape(
                NDT, 128, RQ * 128)).astype(bf)
        expb_col = np.full((128, 1), EXPB, np.float32)
        cfsv = np.zeros((1, 641), np.float32)
        cfsv[0, 0] = EPS
        cfsv[0, 1:129] = 1.0
        cfsv[0, 129:133] = (np.abs(q_gain[RQ * g: RQ * (g + 1)])
                            / np.sqrt(HD)).astype(np.float32)
        cfsv[0, 133:137] = glog[RQ * g: RQ * (g + 1)]
        in_maps.append({
            "xT": xT[b],
            "wqT": np.ascontiguousarray(
                wq.reshape(NDT, 128, RQ * HD).transpose(1, 0, 2)).astype(bf),
            "wkT": np.ascontiguousarray(
                wk_.reshape(NDT, 128, HD).transpose(1, 0, 2)).astype(bf),
            "wvT": np.ascontiguousarray(
                wv.reshape(NDT, 128, HD).transpose(1, 0, 2)).astype(bf),
            "wpT": wpT,
            "cosF": cosF, "sinF": sinF, "cfs": cfsv,
            "onescol_f": onescol_f, "onescol_b": onescol_b,
            "ones8": ones8, "idtr": idtr, "expb": expb_col,
        })
    return in_maps


def kernel(x, Wq, Wk, Wv, Wproj, q_gain, _n_timing_iters=1, _return_raw=False,
           _trace=False):
    x = np.asarray(x, np.float32)
    in_maps = _host_inputs(np.asarray(x, np.float32),
                           np.asarray(Wq, np.float32),
                           np.asarray(Wk, np.float32),
                           np.asarray(Wv, np.float32),
                           np.asarray(Wproj, np.float32),
                           np.asarray(q_gain, np.float32))
    nc = _get_program(_n_timing_iters)
    res = run_bass_kernel_spmd(nc, in_maps, core_ids=list(range(NCORES)),
                               trace=_trace)
    if _return_raw:
        return res
    out = np.zeros((B, S, D), np.float32)
    for c in range(NCORES):
        b = c // HKV
        outT = res.results[c]["outT"].astype(np.float32).reshape(D, S)
        out[b] += outT.T
    return out


if __name__ == "__main__":
    rng = np.random.default_rng(0)
    x = rng.standard_normal((B, S, D)).astype(np.float32)
    Wq = (rng.standard_normal((D, D)) * 0.02).astype(np.float32)
    Wk = (rng.standard_normal((512, D)) * 0.02).astype(np.float32)
    Wv = (rng.standard_normal((512, D)) * 0.02).astype(np.float32)
    Wp = (rng.standard_normal((D, D)) * 0.02).astype(np.float32)
    g = np.ones(H, np.float32)
    out = kernel(x, Wq, Wk, Wv, Wp, g)
    print("out", out.shape, out.dtype, float(np.abs(out).max()))


# revision 11
# speedup vs baseline: 1.2884x; 1.2884x over previous
"""Causal self-attention (GQA + RMS-norm + RoPE) Trainium2 Bass kernel.

Sharding over 8 NeuronCores: 2-way data parallel (batch) x 4-way head
parallel (one GQA group of 4 q-heads + 1 kv-head per core).  Each core
computes q/k/v projections for its group, flash-style causal attention
(scores kept transposed [k, q] so softmax sums ride the tensor engine),
and a partial output projection.  Host sums the 4 per-group partials per
batch.

Engine-balance notes (from NTFF traces):
 - All activations stay in one ACT table set (natural_log_exp_and_others):
   RMS-norm rsqrt and the softmax 1/d are exp(-ln(x)) instead of
   sqrt/reciprocal.  DVE's iterative-divide reciprocal costs ~3.3us per
   [1,512] row; ln+exp on ACT is ~1.4us and avoids table switching.
 - RoPE stages are bf16 so the DVE tensor-tensor ops run in 2x mode.
 - x loads are column-major (j-block at a time) so the first projection
   chains start ~12us into the kernel instead of after the full 8MB load.
 - Off-diagonal attention tiles run the PV and denominator matmuls in
   fp8 DoubleRow pairs (2 k-tiles per instruction); exp emits P in fp8
   with bias -5 (cancels in y/d).  Verified <=4.2e-3 rel err vs 2e-2 gate.
 - Output partials are bf16 (halves the store and the host gather).
"""

import numpy as np
import ml_dtypes

import concourse.bacc as bacc
import concourse.mybir as mybir
from concourse.tile import TileContext
from concourse.bass_utils import run_bass_kernel_spmd
from concourse.hw_specs import get_activation_tables as _get_act_tables

# Pin every activation to the one table set that covers exp/ln/square/copy.
# The insertion pass picks the first set containing each function, which
# otherwise thrashes between exp_and_others and natural_log (one ~2.7us
# ACT_TABLE_LOAD per ln<->exp alternation, ~73 loads per kernel).  Emptying
# the other sets (order/IDs preserved) forces a single hoisted load.
_PIN_SET = "natural_log_exp_and_others"


def _pinned_act_tables(arch):
    tables = _get_act_tables(arch)
    return {name: (fns if name == _PIN_SET else set())
            for name, fns in tables.items()}


bacc.get_activation_tables = _pinned_act_tables

BF16 = mybir.dt.bfloat16
F32 = mybir.dt.float32
F32R = mybir.dt.float32r
FP8 = mybir.dt.float8e4
AF = mybir.ActivationFunctionType
DR = mybir.MatmulPerfMode.DoubleRow
bf = ml_dtypes.bfloat16
f8 = ml_dtypes.float8_e4m3

B, S, D = 2, 2048, 2048
H, HKV, HD = 16, 4, 128
RQ = H // HKV            # q heads per kv group (4)
NCORES = 8
NDT = D // 128           # 16 contraction tiles
NST = S // 512           # 4 query/sequence 512-tiles
NKT = S // 128           # 16 key 128-tiles
EPS = float(np.finfo(np.float32).eps)
EXPB = -5.0              # softmax exp bias: P = e^(s-5); cancels in y/d

_PROG_CACHE = {}


def _build_program(n_timing_iters=1, phases="full"):
    nc = bacc.Bacc("TRN2", debug=False, enable_asserts=False, num_devices=NCORES)

    xT_d = nc.dram_tensor("xT", [128, NDT, S], BF16, kind="ExternalInput")
    wqT_d = nc.dram_tensor("wqT", [128, NDT, RQ * HD], BF16, kind="ExternalInput")
    wkT_d = nc.dram_tensor("wkT", [128, NDT, HD], BF16, kind="ExternalInput")
    wvT_d = nc.dram_tensor("wvT", [128, NDT, HD], BF16, kind="ExternalInput")
    wpT_d = nc.dram_tensor("wpT", [NDT, 128, RQ * 128], BF16, kind="ExternalInput")
    cosF_d = nc.dram_tensor("cosF", [128, S], BF16, kind="ExternalInput")
    sinF_d = nc.dram_tensor("sinF", [128, S], BF16, kind="ExternalInput")
    cfs_d = nc.dram_tensor("cfs", [1, 641], F32R, kind="ExternalInput")
    onescol_f_d = nc.dram_tensor("onescol_f", [128, 1], F32R, kind="ExternalInput")
    expb_d = nc.dram_tensor("expb", [128, 1], F32R, kind="ExternalInput")
    onescol_b_d = nc.dram_tensor("onescol_b", [128, 1], BF16, kind="ExternalInput")
    ones8_d = nc.dram_tensor("ones8", [128, 32], FP8, kind="ExternalInput")
    idtr_d = nc.dram_tensor("idtr", [128, 256], BF16, kind="ExternalInput")
    outT_d = nc.dram_tensor("outT", [NDT, 128, S], BF16, kind="ExternalOutput")

    with TileContext(nc) as tc:
        with tc.tile_pool(name="res", bufs=1) as res, \
             tc.tile_pool(name="work", bufs=2) as wk, \
             tc.tile_pool(name="pwork", bufs=2, space="PSUM") as pw:

            # ---- resident tiles (allocated once) ----
            xT = res.tile([128, NDT, S], BF16)             # [d-part, dt, s]
            wqT = res.tile([128, NDT, RQ * HD], BF16)
            wkT = res.tile([128, NDT, HD], BF16)
            wvT = res.tile([128, NDT, HD], BF16)
            cosF = res.tile([128, S], BF16)
            sinF = res.tile([128, S], BF16)
            cfs = res.tile([1, 641], F32R)
            onescol_f = res.tile([128, 1], F32R)
            expb = res.tile([128, 1], F32R)
            onescol_b = res.tile([128, 1], BF16)
            ones8 = res.tile([128, 2, 16], FP8)
            idtr = res.tile([128, 256], BF16)
            kT = res.tile([128, S], BF16)
            qT = [res.tile([128, S], BF16, name=f"qT{h}", tag=f"qT{h}")
                  for h in range(RQ)]
            yT = [res.tile([128, S], BF16, name=f"yT{h}", tag=f"yT{h}")
                  for h in range(RQ)]
            vTst = res.tile([128, S], BF16)                # v^T staging
            V_all = res.tile([128, S], BF16)               # v natural, kt-major
            V8 = res.tile([128, NKT, 128], FP8)            # fp8 copy of V_all

            eps_ap = cfs[0:1, 0:1].bitcast(F32)
            expb_ap = expb[:, 0:1].bitcast(F32)
            ident = idtr[:, 0:128]
            triu = idtr[:, 128:256]

            def body(_iv=None):
                # ---- load residents ----
                # small constants + k/v weights first (first chains), then x
                # column-major per j-block so chain j can start as soon as
                # its block lands, then q weights (needed ~30us in).
                nc.sync.dma_start(wkT[:], wkT_d[:])
                nc.sync.dma_start(wvT[:], wvT_d[:])
                nc.sync.dma_start(cfs[:], cfs_d[:])
                nc.sync.dma_start(onescol_f[:], onescol_f_d[:])
                nc.sync.dma_start(expb[:], expb_d[:])
                nc.sync.dma_start(onescol_b[:], onescol_b_d[:])
                nc.sync.dma_start(ones8[:], ones8_d[:])
                nc.sync.dma_start(idtr[:], idtr_d[:])
                nc.sync.dma_start(cosF[:], cosF_d[:])
                nc.sync.dma_start(sinF[:], sinF_d[:])
                for j in range(NST):
                    sl = slice(512 * j, 512 * j + 512)
                    nc.sync.dma_start(xT[:, 0:8, sl], xT_d[:, 0:8, sl])
                    nc.sync.dma_start(xT[:, 8:16, sl], xT_d[:, 8:16, sl])
                nc.sync.dma_start(wqT[:, 0:8, :], wqT_d[:, 0:8, :])
                nc.sync.dma_start(wqT[:, 8:16, :], wqT_d[:, 8:16, :])

                def proj_accum(wt_all, col_off, j, tag="big", tbufs=3):
                    """psum [128,512] = sum_d W[d].T @ xT[d, s-slice]"""
                    acc = pw.tile([128, 512], F32, name="acc", tag=tag,
                                  bufs=tbufs)
                    for dt in range(NDT):
                        nc.tensor.matmul(acc[:],
                                         wt_all[:, dt, col_off:col_off + 128],
                                         xT[:, dt, 512 * j:512 * j + 512],
                                         start=(dt == 0), stop=(dt == NDT - 1))
                    return acc

                def norm_rope_chain(acc, lg_ap, dest, j):
                    """RMS-norm + RoPE + scale; writes dest[:, 512j:+512] bf16.

                    rsqrt (and the q gain) ride ACT as exp(-0.5*ln(ms)+lg) so
                    everything stays in one activation table set.  Stages are
                    bf16 so the DVE rope ops run in 2x mode."""
                    stage = wk.tile([128, 512], BF16, name="stage", tag="stage", bufs=3)
                    nc.scalar.copy(stage[:], acc[:])
                    swap = wk.tile([128, 512], BF16, name="swap", tag="swap", bufs=3)
                    nc.sync.dma_start(swap[0:64, :], stage[64:128, :])
                    nc.sync.dma_start(swap[64:128, :], stage[0:64, :])
                    sq = wk.tile([128, 512], F32R, name="sq", tag="sq", bufs=2)
                    nc.scalar.square(sq[:], stage[:])
                    ms = pw.tile([1, 512], F32, name="ms", tag="bcast", bufs=1)
                    nc.tensor.matmul(ms[:], onescol_f[:],
                                     sq[:], start=True, stop=True)
                    lnms = wk.tile([1, 512], F32, name="lnms", tag="lnms", bufs=2)
                    nc.scalar.activation(lnms[:], ms[:], AF.Ln,
                                         bias=eps_ap, scale=1.0 / HD)
                    rg = wk.tile([1, 512], BF16, name="rg", tag="rg", bufs=2)
                    nc.scalar.activation(rg[:], lnms[:], AF.Exp,
                                         bias=(lg_ap if lg_ap is not None
                                               else 0.0), scale=-0.5)
                    Rb = wk.tile([128, 512], BF16, name="Rb", tag="Rb", bufs=2)
                    nc.gpsimd.partition_broadcast(Rb[:], rg[0:1, :])
                    sl = slice(512 * j, 512 * j + 512)
                    nc.vector.tensor_mul(stage[:], stage[:], cosF[:, sl])
                    nc.vector.tensor_mul(swap[:], swap[:], sinF[:, sl])
                    nc.vector.tensor_add(stage[:], stage[:], swap[:])
                    nc.vector.tensor_mul(dest[:, sl], stage[:], Rb[:])

                # ---- projections: mm-chains pipelined one ahead of stats ----
                proj_jobs = []   # (kind, h, j) interleaved to match x arrival
                for j in range(NST):
                    proj_jobs.append(("k", 0, j))
                    proj_jobs.append(("v", 0, j))

                def emit_proj_tail(kind, h, j, acc):
                    if kind == "k":
                        norm_rope_chain(acc, None, kT, j)
                    elif kind == "q":
                        lg_h = cfs[0:1, 133 + h:134 + h].bitcast(F32)
                        norm_rope_chain(acc, lg_h, qT[h], j)
                    else:
                        nc.scalar.copy(vTst[:, 512 * j:512 * j + 512], acc[:])

                def run_proj_jobs(jobs, pending, tags=None, lag=1):
                    for idx, (kind, h, j) in enumerate(jobs):
                        tag, tbufs = ("big", 3) if tags is None else tags[idx]
                        if kind == "k":
                            acc = proj_accum(wkT, 0, j, tag, tbufs)
                        elif kind == "v":
                            acc = proj_accum(wvT, 0, j, tag, tbufs)
                        else:
                            acc = proj_accum(wqT, 128 * h, j, tag, tbufs)
                        pending.append((kind, h, j, acc))
                        if len(pending) > lag:
                            emit_proj_tail(*pending.pop(0))
                    return pending

                # first wave: 7-deep psum concurrency so the x-load DMA
                # wavefront keeps PE fed (every arriving x tile unlocks mms)
                kv_tags = [("big", 3), ("big", 3), ("big", 3), ("acc", 2),
                           ("acc", 2), ("small", 2), ("small", 2), ("big", 3)]
                pending = run_proj_jobs(proj_jobs, [], tags=kv_tags, lag=6)

                # ---- v^T -> V transposes (PE); V kept in bf16 and fp8 ----
                while pending:
                    emit_proj_tail(*pending.pop(0))
                for kt in range(NKT):
                    tp = pw.tile([128, 128], BF16, name="tp", tag="acc", bufs=2)
                    nc.tensor.transpose(tp[:], vTst[:, 128 * kt:128 * kt + 128],
                                        ident)
                    nc.scalar.copy(V_all[:, 128 * kt:128 * kt + 128], tp[:])
                    nc.scalar.copy(V8[:, kt, :], tp[:])

                if phases == "kv":
                    return

                def attention_block(h, j):
                    """Causal attention for queries [512j, 512j+512), head h.

                    Off-diagonal k-tiles (kt < 4j) are consumed as fp8
                    DoubleRow pairs (P in fp8 from exp, V from V8): one PV and
                    one denominator matmul per TWO k-tiles.  Diagonal tiles
                    stay bf16 with column trimming + triu mask.  Consumers
                    trail the S-matmul/exp pipeline so PE never waits on ACT."""
                    nkt = 4 * j + 4
                    npair = (4 * j) // 2
                    acc_y = pw.tile([128, 512], F32, name="acc_y", tag="acc",
                                    bufs=2)
                    acc_d = pw.tile([1, 512], F32, name="acc_d", tag="small",
                                    bufs=2)
                    ncons = npair + 4
                    lagged = []

                    def consume(ci, kind, P, c0):
                        first, last = ci == 0, ci == ncons - 1
                        if kind == "pair":
                            nc.tensor.matmul(acc_d[:], ones8[:, 0:2, 0:1],
                                             P[:, 0:2, :],
                                             start=first, stop=last,
                                             perf_mode=DR,
                                             skip_group_check=True)
                            nc.tensor.matmul(acc_y[:], V8[:, c0:c0 + 2, :],
                                             P[:, 0:2, :],
                                             start=first, stop=last,
                                             perf_mode=DR,
                                             skip_group_check=True)
                        else:
                            kt = c0
                            cc = 128 * (kt - 4 * j)
                            nc.tensor.matmul(acc_d[0:1, cc:512], onescol_b[:],
                                             P[:, cc:512],
                                             start=first, stop=last,
                                             skip_group_check=True)
                            nc.tensor.matmul(acc_y[:, cc:512],
                                             V_all[:, 128 * kt:128 * kt + 128],
                                             P[:, cc:512],
                                             start=first, stop=last,
                                             skip_group_check=True)

                    ci = 0
                    for pi in range(npair):
                        P8 = wk.tile([128, 2, 512], FP8, name="P8", tag="P8",
                                     bufs=4)
                        for u in (0, 1):
                            kt = 2 * pi + u
                            ps = pw.tile([128, 512], F32, name="ps", tag="big",
                                         bufs=3)
                            nc.tensor.matmul(
                                ps[:], kT[:, 128 * kt:128 * kt + 128],
                                qT[h][:, 512 * j:512 * j + 512],
                                start=True, stop=True)
                            nc.scalar.activation(P8[:, u, :], ps[:], AF.Exp,
                                                 bias=expb_ap)
                        lagged.append((ci, "pair", P8, 2 * pi))
                        ci += 1
                        if len(lagged) > 2:
                            consume(*lagged.pop(0))
                    for kt in range(4 * j, nkt):
                        cc = 128 * (kt - 4 * j)
                        ps = pw.tile([128, 512], F32, name="ps", tag="big",
                                     bufs=3)
                        nc.tensor.matmul(
                            ps[:, cc:512],
                            kT[:, 128 * kt:128 * kt + 128],
                            qT[h][:, 512 * j + cc:512 * j + 512],
                            start=True, stop=True)
                        P = wk.tile([128, 512], BF16, name="P", tag="P", bufs=4)
                        nc.scalar.activation(P[:, cc:512], ps[:, cc:512],
                                             AF.Exp, bias=expb_ap)
                        nc.vector.tensor_mul(P[:, cc:cc + 128],
                                             P[:, cc:cc + 128], triu)
                        lagged.append((ci, "diag", P, kt))
                        ci += 1
                        if len(lagged) > 2:
                            consume(*lagged.pop(0))
                    while lagged:
                        consume(*lagged.pop(0))
                    # 1/d = exp(-ln(d)) on ACT (stays in the same table set)
                    lnd = wk.tile([1, 512], F32, name="lnd", tag="lnms", bufs=2)
                    nc.scalar.activation(lnd[:], acc_d[:], AF.Ln)
                    rdv = wk.tile([1, 512], BF16, name="rdv", tag="rg", bufs=2)
                    nc.scalar.activation(rdv[:], lnd[:], AF.Exp, scale=-1.0)
                    Rd = wk.tile([128, 512], BF16, name="Rd", tag="Rb", bufs=2)
                    nc.gpsimd.partition_broadcast(Rd[:], rdv[0:1, :])
                    nc.vector.tensor_mul(yT[h][:, 512 * j:512 * j + 512],
                                         acc_y[:], Rd[:])

                # ---- per q-head: q(h+1) projections emitted ahead of
                # attention(h) so PE crosses head boundaries without gaps ----
                pending = run_proj_jobs([("q", 0, j) for j in range(NST)],
                                        pending)
                for h in range(RQ):
                    if h + 1 < RQ:
                        pending = run_proj_jobs(
                            [("q", h + 1, j) for j in range(NST)], pending)
                    while pending:
                        emit_proj_tail(*pending.pop(0))
                    if phases == "kvq":
                        continue
                    for j in range(NST):
                        attention_block(h, j)

                # ---- output projection (transposed: out^T[D, s]) ----
                if phases in ("kv", "kvq", "noout"):
                    return
                ptags = ["big", "acc", "bcast", "small"]
                pbufs = {"big": 3, "acc": 2, "bcast": 1, "small": 2}
                for dt in range(NDT):
                    wp = wk.tile([128, RQ * 128], BF16, name="wp", tag="wp",
                                 bufs=3)
                    nc.sync.dma_start(wp[:], wpT_d[dt])
                    osb = wk.tile([128, S], BF16, name="osb", tag="osb")
                    for sjj in range(NST):
                        po = pw.tile([128, 512], F32, name=f"po{sjj}",
                                     tag=ptags[sjj], bufs=pbufs[ptags[sjj]])
                        for h in range(RQ):
                            nc.tensor.matmul(
                                po[:], wp[:, 128 * h:128 * h + 128],
                                yT[h][:, 512 * sjj:512 * sjj + 512],
                                start=(h == 0), stop=(h == RQ - 1))
                        # psum->sbuf copies split across ACT and DVE
                        if sjj % 2 == 0:
                            nc.scalar.copy(osb[:, 512 * sjj:512 * sjj + 512],
                                           po[:])
                        else:
                            nc.vector.tensor_copy(
                                osb[:, 512 * sjj:512 * sjj + 512], po[:])
                    nc.sync.dma_start(outT_d[dt], osb[:])

            if n_timing_iters > 1:
                with tc.For_i(0, n_timing_iters, 1):
                    body()
            else:
                body()

    nc.compile()
    return nc


def _get_program(n_timing_iters=1, phases="full"):
    key = (n_timing_iters, phases)
    if key not in _PROG_CACHE:
        _PROG_CACHE[key] = _build_program(n_timing_iters, phases)
    return _PROG_CACHE[key]


def _host_inputs(x, Wq, Wk, Wv, Wproj, q_gain):
    """Build the 8 per-core input maps (host-side layout prep)."""
    inv = 1.0 / (10000.0 ** (np.arange(0, HD, 2, dtype=np.float64) / HD))
    t = np.arange(S, dtype=np.float64)
    fr = np.outer(t, inv).astype(np.float32)          # [S, 64]
    cos = np.cos(fr).astype(np.float32)
    sin = np.sin(fr).astype(np.float32)
    cosF = np.concatenate([cos.T, cos.T], 0).astype(bf)          # [128, S]
    sinF = np.concatenate([sin.T, -sin.T], 0).astype(bf)

    onescol_f = np.ones((128, 1), np.float32)
    onescol_b = np.ones((128, 1), bf)
    ones8 = np.ones((128, 32), f8)
    ident = np.eye(128, dtype=np.float32)
    triu = (np.arange(128)[None, :] >= np.arange(128)[:, None]).astype(np.float32)
    idtr = np.concatenate([ident, triu], 1).astype(bf)

    # fold gain sign into Wq rows; |gain| rides the rsqrt exp as a ln-bias
    gsign = np.where(q_gain < 0, -1.0, 1.0).astype(np.float32)
    Wq = Wq * np.repeat(gsign, HD)[:, None]
    glog = np.log(np.maximum(np.abs(q_gain.astype(np.float64)), 1e-300)
                  / np.sqrt(HD))
    glog = np.maximum(glog, -80.0).astype(np.float32)

    # [128, NDT, S]: xT[p, dt, s] = x[b][s, 128*dt+p]
    xT = [np.ascontiguousarray(
        x[b].T.reshape(NDT, 128, S).transpose(1, 0, 2)).astype(bf)
        for b in range(B)]

    in_maps = []
    for c in range(NCORES):
        b, g = c // HKV, c % HKV
        wq = np.ascontiguousarray(Wq[512 * g:512 * (g + 1)].T)   # [D, 512]
        wk_ = np.ascontiguousarray(Wk[128 * g:128 * (g + 1)].T)  # [D, 128]
        wv = np.ascontiguousarray(Wv[128 * g:128 * (g + 1)].T)
        wpT = np.ascontiguousarray(Wproj[:, 512 * g:512 * (g + 1)].T)  # [512, 2048]
        # [dt][c-part 128, (h, m) 512]: wpT2[dt, c, 128h+m] = Wp[128dt+m, 512g+128h+c]
        wpT = np.ascontiguousarray(
            wpT.reshape(RQ, 128, NDT, 128).transpose(2, 1, 0, 3).reshape(
                NDT, 128, RQ * 128)).astype(bf)
        expb_col = np.full((128, 1), EXPB, np.float32)
        cfsv = np.zeros((1, 641), np.float32)
        cfsv[0, 0] = EPS
        cfsv[0, 1:129] = 1.0
        cfsv[0, 129:133] = (np.abs(q_gain[RQ * g: RQ * (g + 1)])
                            / np.sqrt(HD)).astype(np.float32)
        cfsv[0, 133:137] = glog[RQ * g: RQ * (g + 1)]
        in_maps.append({
            "xT": xT[b],
            "wqT": np.ascontiguousarray(
                wq.reshape(NDT, 128, RQ * HD).transpose(1, 0, 2)).astype(bf),
            "wkT": np.ascontiguousarray(
                wk_.reshape(NDT, 128, HD).transpose(1, 0, 2)).astype(bf),
            "wvT": np.ascontiguousarray(
                wv.reshape(NDT, 128, HD).transpose(1, 0, 2)).astype(bf),
            "wpT": wpT,
            "cosF": cosF, "sinF": sinF, "cfs": cfsv,
            "onescol_f": onescol_f, "onescol_b": onescol_b,
            "ones8": ones8, "idtr": idtr, "expb": expb_col,
        })
    return in_maps


def kernel(x, Wq, Wk, Wv, Wproj, q_gain, _n_timing_iters=1, _return_raw=False,
           _trace=False):
    x = np.asarray(x, np.float32)
    in_maps = _host_inputs(np.asarray(x, np.float32),
                           np.asarray(Wq, np.float32),
                           np.asarray(Wk, np.float32),
                           np.asarray(Wv, np.float32),
                           np.asarray(Wproj, np.float32),
                           np.asarray(q_gain, np.float32))
    nc = _get_program(_n_timing_iters)
    res = run_bass_kernel_spmd(nc, in_maps, core_ids=list(range(NCORES)),
                               trace=_trace)
    if _return_raw:
        return res
    out = np.zeros((B, S, D), np.float32)
    for c in range(NCORES):
        b = c // HKV
        outT = res.results[c]["outT"].astype(np.float32).reshape(D, S)
        out[b] += outT.T
    return out


if __name__ == "__main__":
    rng = np.random.default_rng(0)
    x = rng.standard_normal((B, S, D)).astype(np.float32)
    Wq = (rng.standard_normal((D, D)) * 0.02).astype(np.float32)
    Wk = (rng.standard_normal((512, D)) * 0.02).astype(np.float32)
    Wv = (rng.standard_normal((512, D)) * 0.02).astype(np.float32)
    Wp = (rng.standard_normal((D, D)) * 0.02).astype(np.float32)
    g = np.ones(H, np.float32)
    out = kernel(x, Wq, Wk, Wv, Wp, g)
    print("out", out.shape, out.dtype, float(np.abs(out).max()))


# revision 15
# speedup vs baseline: 1.3115x; 1.0179x over previous
"""Causal self-attention (GQA + RMS-norm + RoPE) Trainium2 Bass kernel.

Sharding over 8 NeuronCores: 2-way data parallel (batch) x 4-way head
parallel (one GQA group of 4 q-heads + 1 kv-head per core).  Each core
computes q/k/v projections for its group, flash-style causal attention
(scores kept transposed [k, q] so softmax sums ride the tensor engine),
and a partial output projection.  Host sums the 4 per-group partials per
batch.

Engine-balance notes (from NTFF traces):
 - All activations stay in one ACT table set (natural_log_exp_and_others):
   RMS-norm rsqrt and the softmax 1/d are exp(-ln(x)) instead of
   sqrt/reciprocal.  DVE's iterative-divide reciprocal costs ~3.3us per
   [1,512] row; ln+exp on ACT is ~1.4us and avoids table switching.
 - RoPE stages are bf16 so the DVE tensor-tensor ops run in 2x mode.
 - x loads are column-major (j-block at a time) so the first projection
   chains start ~12us into the kernel instead of after the full 8MB load.
 - Off-diagonal attention tiles run the PV and denominator matmuls in
   fp8 DoubleRow pairs (2 k-tiles per instruction); exp emits P in fp8
   with bias -5 (cancels in y/d).  Verified <=4.2e-3 rel err vs 2e-2 gate.
 - Output partials are bf16 (halves the store and the host gather).
"""

import numpy as np
import ml_dtypes

import concourse.bacc as bacc
import concourse.mybir as mybir
from concourse.tile import TileContext
from concourse.bass_utils import run_bass_kernel_spmd
from concourse.hw_specs import get_activation_tables as _get_act_tables

# Pin every activation to the one table set that covers exp/ln/square/copy.
# The insertion pass picks the first set containing each function, which
# otherwise thrashes between exp_and_others and natural_log (one ~2.7us
# ACT_TABLE_LOAD per ln<->exp alternation, ~73 loads per kernel).  Emptying
# the other sets (order/IDs preserved) forces a single hoisted load.
_PIN_SET = "natural_log_exp_and_others"


def _pinned_act_tables(arch):
    tables = _get_act_tables(arch)
    return {name: (fns if name == _PIN_SET else set())
            for name, fns in tables.items()}


bacc.get_activation_tables = _pinned_act_tables

BF16 = mybir.dt.bfloat16
F32 = mybir.dt.float32
F32R = mybir.dt.float32r
FP8 = mybir.dt.float8e4
AF = mybir.ActivationFunctionType
DR = mybir.MatmulPerfMode.DoubleRow
bf = ml_dtypes.bfloat16
f8 = ml_dtypes.float8_e4m3

B, S, D = 2, 2048, 2048
H, HKV, HD = 16, 4, 128
RQ = H // HKV            # q heads per kv group (4)
NCORES = 8
NDT = D // 128           # 16 contraction tiles
NST = S // 512           # 4 query/sequence 512-tiles
NKT = S // 128           # 16 key 128-tiles
EPS = float(np.finfo(np.float32).eps)
EXPB = -5.0              # softmax exp bias: P = e^(s-5); cancels in y/d

_PROG_CACHE = {}


def _build_program(n_timing_iters=1, phases="full"):
    nc = bacc.Bacc("TRN2", debug=False, enable_asserts=False, num_devices=NCORES)

    xT_d = nc.dram_tensor("xT", [128, NDT, S], BF16, kind="ExternalInput")
    wqT_d = nc.dram_tensor("wqT", [128, NDT, RQ * HD], BF16, kind="ExternalInput")
    wkT_d = nc.dram_tensor("wkT", [128, NDT, HD], BF16, kind="ExternalInput")
    wvT_d = nc.dram_tensor("wvT", [128, NDT, HD], BF16, kind="ExternalInput")
    wpT_d = nc.dram_tensor("wpT", [NDT, 128, RQ * 128], BF16, kind="ExternalInput")
    cosF_d = nc.dram_tensor("cosF", [128, S], BF16, kind="ExternalInput")
    sinF_d = nc.dram_tensor("sinF", [128, S], BF16, kind="ExternalInput")
    cfs_d = nc.dram_tensor("cfs", [1, 641], F32R, kind="ExternalInput")
    onescol_f_d = nc.dram_tensor("onescol_f", [128, 1], F32R, kind="ExternalInput")
    expb_d = nc.dram_tensor("expb", [128, 1], F32R, kind="ExternalInput")
    onescol_b_d = nc.dram_tensor("onescol_b", [128, 1], BF16, kind="ExternalInput")
    ones8_d = nc.dram_tensor("ones8", [128, 32], FP8, kind="ExternalInput")
    idtr_d = nc.dram_tensor("idtr", [128, 256], BF16, kind="ExternalInput")
    outT_d = nc.dram_tensor("outT", [NDT, 128, S], BF16, kind="ExternalOutput")

    with TileContext(nc) as tc:
        with tc.tile_pool(name="res", bufs=1) as res, \
             tc.tile_pool(name="work", bufs=2) as wk, \
             tc.tile_pool(name="pwork", bufs=2, space="PSUM") as pw:

            # ---- resident tiles (allocated once) ----
            xT = res.tile([128, NDT, S], BF16)             # [d-part, dt, s]
            wqT = res.tile([128, NDT, RQ * HD], BF16)
            wkT = res.tile([128, NDT, HD], BF16)
            wvT = res.tile([128, NDT, HD], BF16)
            cosF = res.tile([128, S], BF16)
            sinF = res.tile([128, S], BF16)
            cfs = res.tile([1, 641], F32R)
            onescol_f = res.tile([128, 1], F32R)
            expb = res.tile([128, 1], F32R)
            onescol_b = res.tile([128, 1], BF16)
            ones8 = res.tile([128, 2, 16], FP8)
            idtr = res.tile([128, 256], BF16)
            kT = res.tile([128, S], BF16)
            qT = [res.tile([128, S], BF16, name=f"qT{h}", tag=f"qT{h}")
                  for h in range(RQ)]
            yT = [res.tile([128, S], BF16, name=f"yT{h}", tag=f"yT{h}")
                  for h in range(RQ)]
            vTst = res.tile([128, S], BF16)                # v^T staging
            V_all = res.tile([128, S], BF16)               # v natural, kt-major
            V8 = res.tile([128, NKT, 128], FP8)            # fp8 copy of V_all

            eps_ap = cfs[0:1, 0:1].bitcast(F32)
            expb_ap = expb[:, 0:1].bitcast(F32)
            ident = idtr[:, 0:128]
            triu = idtr[:, 128:256]

            def body(_iv=None):
                # ---- load residents ----
                # Ordered by first use: k weights + x block 0 gate the first
                # chain (~8us), v weights + later x blocks stream behind,
                # cos/sin only gate the (latency-tolerant) rope tails, q
                # weights are needed ~30us in, attention consts later still.
                nc.sync.dma_start(wkT[:], wkT_d[:])
                nc.sync.dma_start(cfs[:], cfs_d[:])
                nc.sync.dma_start(onescol_f[:], onescol_f_d[:])

                def xblk(j):
                    sl = slice(512 * j, 512 * j + 512)
                    nc.sync.dma_start(xT[:, 0:8, sl], xT_d[:, 0:8, sl])
                    nc.sync.dma_start(xT[:, 8:16, sl], xT_d[:, 8:16, sl])

                xblk(0)
                nc.sync.dma_start(wvT[:], wvT_d[:])
                xblk(1)
                nc.sync.dma_start(cosF[:], cosF_d[:])
                nc.sync.dma_start(sinF[:], sinF_d[:])
                xblk(2)
                xblk(3)
                nc.sync.dma_start(wqT[:, 0:8, :], wqT_d[:, 0:8, :])
                nc.sync.dma_start(wqT[:, 8:16, :], wqT_d[:, 8:16, :])
                nc.sync.dma_start(expb[:], expb_d[:])
                nc.sync.dma_start(onescol_b[:], onescol_b_d[:])
                nc.sync.dma_start(ones8[:], ones8_d[:])
                nc.sync.dma_start(idtr[:], idtr_d[:])

                def proj_accum(wt_all, col_off, j, tag="big", tbufs=3):
                    """psum [128,512] = sum_d W[d].T @ xT[d, s-slice]"""
                    acc = pw.tile([128, 512], F32, name="acc", tag=tag,
                                  bufs=tbufs)
                    for dt in range(NDT):
                        nc.tensor.matmul(acc[:],
                                         wt_all[:, dt, col_off:col_off + 128],
                                         xT[:, dt, 512 * j:512 * j + 512],
                                         start=(dt == 0), stop=(dt == NDT - 1))
                    return acc

                def norm_rope_chain(acc, lg_ap, dest, j):
                    """RMS-norm + RoPE + scale; writes dest[:, 512j:+512] bf16.

                    rsqrt (and the q gain) ride ACT as exp(-0.5*ln(ms)+lg) so
                    everything stays in one activation table set.  Stages are
                    bf16 so the DVE rope ops run in 2x mode."""
                    stage = wk.tile([128, 512], BF16, name="stage", tag="stage", bufs=3)
                    nc.vector.tensor_copy(stage[:], acc[:])
                    swap = wk.tile([128, 512], BF16, name="swap", tag="swap", bufs=3)
                    nc.sync.dma_start(swap[0:64, :], stage[64:128, :])
                    nc.sync.dma_start(swap[64:128, :], stage[0:64, :])
                    sq = wk.tile([128, 512], F32R, name="sq", tag="sq", bufs=2)
                    nc.scalar.square(sq[:], stage[:])
                    ms = pw.tile([1, 512], F32, name="ms", tag="bcast", bufs=1)
                    nc.tensor.matmul(ms[:], onescol_f[:],
                                     sq[:], start=True, stop=True)
                    lnms = wk.tile([1, 512], F32, name="lnms", tag="lnms", bufs=2)
                    nc.scalar.activation(lnms[:], ms[:], AF.Ln,
                                         bias=eps_ap, scale=1.0 / HD)
                    rg = wk.tile([1, 512], BF16, name="rg", tag="rg", bufs=2)
                    nc.scalar.activation(rg[:], lnms[:], AF.Exp,
                                         bias=(lg_ap if lg_ap is not None
                                               else 0.0), scale=-0.5)
                    Rb = wk.tile([128, 512], BF16, name="Rb", tag="Rb", bufs=2)
                    nc.gpsimd.partition_broadcast(Rb[:], rg[0:1, :])
                    sl = slice(512 * j, 512 * j + 512)
                    nc.vector.tensor_mul(stage[:], stage[:], cosF[:, sl])
                    nc.vector.tensor_mul(swap[:], swap[:], sinF[:, sl])
                    nc.vector.tensor_add(stage[:], stage[:], swap[:])
                    nc.vector.tensor_mul(dest[:, sl], stage[:], Rb[:])

                # ---- projections: mm-chains pipelined one ahead of stats ----
                proj_jobs = []   # (kind, h, j) interleaved to match x arrival
                for j in range(NST):
                    proj_jobs.append(("k", 0, j))
                    proj_jobs.append(("v", 0, j))

                def emit_proj_tail(kind, h, j, acc):
                    if kind == "k":
                        norm_rope_chain(acc, None, kT, j)
                    elif kind == "q":
                        lg_h = cfs[0:1, 133 + h:134 + h].bitcast(F32)
                        norm_rope_chain(acc, lg_h, qT[h], j)
                    else:
                        nc.scalar.copy(vTst[:, 512 * j:512 * j + 512], acc[:])

                def run_proj_jobs(jobs, pending, tags=None, lag=1):
                    for idx, (kind, h, j) in enumerate(jobs):
                        tag, tbufs = ("big", 3) if tags is None else tags[idx]
                        if kind == "k":
                            acc = proj_accum(wkT, 0, j, tag, tbufs)
                        elif kind == "v":
                            acc = proj_accum(wvT, 0, j, tag, tbufs)
                        else:
                            acc = proj_accum(wqT, 128 * h, j, tag, tbufs)
                        pending.append((kind, h, j, acc))
                        if len(pending) > lag:
                            emit_proj_tail(*pending.pop(0))
                    return pending

                # first wave: 7-deep psum concurrency so the x-load DMA
                # wavefront keeps PE fed (every arriving x tile unlocks mms)
                kv_tags = [("big", 3), ("big", 3), ("big", 3), ("acc", 2),
                           ("acc", 2), ("small", 2), ("small", 2), ("big", 3)]
                pending = run_proj_jobs(proj_jobs, [], tags=kv_tags, lag=6)

                # ---- v^T -> V transposes (PE); V kept in bf16 and fp8 ----
                while pending:
                    emit_proj_tail(*pending.pop(0))
                for kt in range(NKT):
                    tp = pw.tile([128, 128], BF16, name="tp", tag="acc", bufs=2)
                    nc.tensor.transpose(tp[:], vTst[:, 128 * kt:128 * kt + 128],
                                        ident)
                    nc.scalar.copy(V_all[:, 128 * kt:128 * kt + 128], tp[:])
                    nc.vector.tensor_copy(V8[:, kt, :], tp[:])

                if phases == "kv":
                    return

                def attention_block(h, j):
                    """Causal attention for queries [512j, 512j+512), head h.

                    Off-diagonal k-tiles (kt < 4j) are consumed as fp8
                    DoubleRow pairs (P in fp8 from exp, V from V8): one PV and
                    one denominator matmul per TWO k-tiles.  Diagonal tiles
                    stay bf16 with column trimming + triu mask.  Consumers
                    trail the S-matmul/exp pipeline so PE never waits on ACT."""
                    nkt = 4 * j + 4
                    npair = (4 * j) // 2
                    acc_y = pw.tile([128, 512], F32, name="acc_y", tag="acc",
                                    bufs=2)
                    acc_d = pw.tile([1, 512], F32, name="acc_d", tag="small",
                                    bufs=2)
                    ncons = npair + 4
                    lagged = []

                    def consume(ci, kind, P, c0):
                        first, last = ci == 0, ci == ncons - 1
                        if kind == "pair":
                            nc.tensor.matmul(acc_d[:], ones8[:, 0:2, 0:1],
                                             P[:, 0:2, :],
                                             start=first, stop=last,
                                             perf_mode=DR,
                                             skip_group_check=True)
                            nc.tensor.matmul(acc_y[:], V8[:, c0:c0 + 2, :],
                                             P[:, 0:2, :],
                                             start=first, stop=last,
                                             perf_mode=DR,
                                             skip_group_check=True)
                        else:
                            kt = c0
                            cc = 128 * (kt - 4 * j)
                            nc.tensor.matmul(acc_d[0:1, cc:512], onescol_b[:],
                                             P[:, cc:512],
                                             start=first, stop=last,
                                             skip_group_check=True)
                            nc.tensor.matmul(acc_y[:, cc:512],
                                             V_all[:, 128 * kt:128 * kt + 128],
                                             P[:, cc:512],
                                             start=first, stop=last,
                                             skip_group_check=True)

                    # interleave pair and diag units: the pair phase is
                    # ACT-bound (two 512-wide exps per ~0.9us of PE) while the
                    # diag phase has ACT slack, so alternating keeps both fed.
                    units = []
                    pu, dk = 0, 4 * j
                    while pu < npair or dk < nkt:
                        if pu < npair:
                            units.append(("pair", pu))
                            pu += 1
                        if dk < nkt:
                            units.append(("diag", dk))
                            dk += 1
                    ci = 0
                    for kind, idx in units:
                        if kind == "pair":
                            P8 = wk.tile([128, 2, 512], FP8, name="P8",
                                         tag="P8", bufs=4)
                            for u in (0, 1):
                                kt = 2 * idx + u
                                ps = pw.tile([128, 512], F32, name="ps",
                                             tag="big", bufs=3)
                                nc.tensor.matmul(
                                    ps[:], kT[:, 128 * kt:128 * kt + 128],
                                    qT[h][:, 512 * j:512 * j + 512],
                                    start=True, stop=True)
                                nc.scalar.activation(P8[:, u, :], ps[:],
                                                     AF.Exp, bias=expb_ap)
                            lagged.append((ci, "pair", P8, 2 * idx))
                        else:
                            kt = idx
                            cc = 128 * (kt - 4 * j)
                            ps = pw.tile([128, 512], F32, name="ps", tag="big",
                                         bufs=3)
                            nc.tensor.matmul(
                                ps[:, cc:512],
                                kT[:, 128 * kt:128 * kt + 128],
                                qT[h][:, 512 * j + cc:512 * j + 512],
                                start=True, stop=True)
                            P = wk.tile([128, 512], BF16, name="P", tag="P",
                                        bufs=4)
                            nc.scalar.activation(P[:, cc:512], ps[:, cc:512],
                                                 AF.Exp, bias=expb_ap)
                            nc.vector.tensor_mul(P[:, cc:cc + 128],
                                                 P[:, cc:cc + 128], triu)
                            lagged.append((ci, "diag", P, kt))
                        ci += 1
                        if len(lagged) > 2:
                            consume(*lagged.pop(0))
                    while lagged:
                        consume(*lagged.pop(0))
                    # 1/d = exp(-ln(d)) on ACT (stays in the same table set)
                    lnd = wk.tile([1, 512], F32, name="lnd", tag="lnms", bufs=2)
                    nc.scalar.activation(lnd[:], acc_d[:], AF.Ln)
                    rdv = wk.tile([1, 512], BF16, name="rdv", tag="rg", bufs=2)
                    nc.scalar.activation(rdv[:], lnd[:], AF.Exp, scale=-1.0)
                    Rd = wk.tile([128, 512], BF16, name="Rd", tag="Rb", bufs=2)
                    nc.gpsimd.partition_broadcast(Rd[:], rdv[0:1, :])
                    nc.vector.tensor_mul(yT[h][:, 512 * j:512 * j + 512],
                                         acc_y[:], Rd[:])

                # ---- per q-head: q(h+1) projections emitted ahead of
                # attention(h) so PE crosses head boundaries without gaps ----
                pending = run_proj_jobs([("q", 0, j) for j in range(NST)],
                                        pending)
                for h in range(RQ):
                    if h + 1 < RQ:
                        pending = run_proj_jobs(
                            [("q", h + 1, j) for j in range(NST)], pending)
                    while pending:
                        emit_proj_tail(*pending.pop(0))
                    if phases == "kvq":
                        continue
                    for j in range(NST):
                        attention_block(h, j)

                # ---- output projection (transposed: out^T[D, s]) ----
                if phases in ("kv", "kvq", "noout"):
                    return
                ptags = ["big", "acc", "bcast", "small"]
                pbufs = {"big": 3, "acc": 2, "bcast": 1, "small": 2}
                for dt in range(NDT):
                    wp = wk.tile([128, RQ * 128], BF16, name="wp", tag="wp",
                                 bufs=3)
                    nc.sync.dma_start(wp[:], wpT_d[dt])
                    osb = wk.tile([128, S], BF16, name="osb", tag="osb")
                    for sjj in range(NST):
                        po = pw.tile([128, 512], F32, name=f"po{sjj}",
                                     tag=ptags[sjj], bufs=pbufs[ptags[sjj]])
                        for h in range(RQ):
                            nc.tensor.matmul(
                                po[:], wp[:, 128 * h:128 * h + 128],
                                yT[h][:, 512 * sjj:512 * sjj + 512],
                                start=(h == 0), stop=(h == RQ - 1))
                        # psum->sbuf copies split across ACT and DVE
                        if sjj % 2 == 0:
                            nc.scalar.copy(osb[:, 512 * sjj:512 * sjj + 512],
                                           po[:])
                        else:
                            nc.vector.tensor_copy(
                                osb[:, 512 * sjj:512 * sjj + 512], po[:])
                    nc.sync.dma_start(outT_d[dt], osb[:])

            if n_timing_iters > 1:
                with tc.For_i(0, n_timing_iters, 1):
                    body()
            else:
                body()

    nc.compile()
    return nc


def _get_program(n_timing_iters=1, phases="full"):
    key = (n_timing_iters, phases)
    if key not in _PROG_CACHE:
        _PROG_CACHE[key] = _build_program(n_timing_iters, phases)
    return _PROG_CACHE[key]


def _host_inputs(x, Wq, Wk, Wv, Wproj, q_gain):
    """Build the 8 per-core input maps (host-side layout prep)."""
    inv = 1.0 / (10000.0 ** (np.arange(0, HD, 2, dtype=np.float64) / HD))
    t = np.arange(S, dtype=np.float64)
    fr = np.outer(t, inv).astype(np.float32)          # [S, 64]
    cos = np.cos(fr).astype(np.float32)
    sin = np.sin(fr).astype(np.float32)
    cosF = np.concatenate([cos.T, cos.T], 0).astype(bf)          # [128, S]
    sinF = np.concatenate([sin.T, -sin.T], 0).astype(bf)

    onescol_f = np.ones((128, 1), np.float32)
    onescol_b = np.ones((128, 1), bf)
    ones8 = np.ones((128, 32), f8)
    ident = np.eye(128, dtype=np.float32)
    triu = (np.arange(128)[None, :] >= np.arange(128)[:, None]).astype(np.float32)
    idtr = np.concatenate([ident, triu], 1).astype(bf)

    # fold gain sign into Wq rows; |gain| rides the rsqrt exp as a ln-bias
    gsign = np.where(q_gain < 0, -1.0, 1.0).astype(np.float32)
    Wq = Wq * np.repeat(gsign, HD)[:, None]
    glog = np.log(np.maximum(np.abs(q_gain.astype(np.float64)), 1e-300)
                  / np.sqrt(HD))
    glog = np.maximum(glog, -80.0).astype(np.float32)

    # [128, NDT, S]: xT[p, dt, s] = x[b][s, 128*dt+p]
    xT = [np.ascontiguousarray(
        x[b].T.reshape(NDT, 128, S).transpose(1, 0, 2)).astype(bf)
        for b in range(B)]

    in_maps = []
    for c in range(NCORES):
        b, g = c // HKV, c % HKV
        wq = np.ascontiguousarray(Wq[512 * g:512 * (g + 1)].T)   # [D, 512]
        wk_ = np.ascontiguousarray(Wk[128 * g:128 * (g + 1)].T)  # [D, 128]
        wv = np.ascontiguousarray(Wv[128 * g:128 * (g + 1)].T)
        wpT = np.ascontiguousarray(Wproj[:, 512 * g:512 * (g + 1)].T)  # [512, 2048]
        # [dt][c-part 128, (h, m) 512]: wpT2[dt, c, 128h+m] = Wp[128dt+m, 512g+128h+c]
        wpT = np.ascontiguousarray(
            wpT.reshape(RQ, 128, NDT, 128).transpose(2, 1, 0, 3).reshape(
                NDT, 128, RQ * 128)).astype(bf)
        expb_col = np.full((128, 1), EXPB, np.float32)
        cfsv = np.zeros((1, 641), np.float32)
        cfsv[0, 0] = EPS
        cfsv[0, 1:129] = 1.0
        cfsv[0, 129:133] = (np.abs(q_gain[RQ * g: RQ * (g + 1)])
                            / np.sqrt(HD)).astype(np.float32)
        cfsv[0, 133:137] = glog[RQ * g: RQ * (g + 1)]
        in_maps.append({
            "xT": xT[b],
            "wqT": np.ascontiguousarray(
                wq.reshape(NDT, 128, RQ * HD).transpose(1, 0, 2)).astype(bf),
            "wkT": np.ascontiguousarray(
                wk_.reshape(NDT, 128, HD).transpose(1, 0, 2)).astype(bf),
            "wvT": np.ascontiguousarray(
                wv.reshape(NDT, 128, HD).transpose(1, 0, 2)).astype(bf),
            "wpT": wpT,
            "cosF": cosF, "sinF": sinF, "cfs": cfsv,
            "onescol_f": onescol_f, "onescol_b": onescol_b,
            "ones8": ones8, "idtr": idtr, "expb": expb_col,
        })
    return in_maps


def kernel(x, Wq, Wk, Wv, Wproj, q_gain, _n_timing_iters=1, _return_raw=False,
           _trace=False):
    x = np.asarray(x, np.float32)
    in_maps = _host_inputs(np.asarray(x, np.float32),
                           np.asarray(Wq, np.float32),
                           np.asarray(Wk, np.float32),
                           np.asarray(Wv, np.float32),
                           np.asarray(Wproj, np.float32),
                           np.asarray(q_gain, np.float32))
    nc = _get_program(_n_timing_iters)
    res = run_bass_kernel_spmd(nc, in_maps, core_ids=list(range(NCORES)),
                               trace=_trace)
    if _return_raw:
        return res
    out = np.zeros((B, S, D), np.float32)
    for c in range(NCORES):
        b = c // HKV
        outT = res.results[c]["outT"].astype(np.float32).reshape(D, S)
        out[b] += outT.T
    return out


if __name__ == "__main__":
    rng = np.random.default_rng(0)
    x = rng.standard_normal((B, S, D)).astype(np.float32)
    Wq = (rng.standard_normal((D, D)) * 0.02).astype(np.float32)
    Wk = (rng.standard_normal((512, D)) * 0.02).astype(np.float32)
    Wv = (rng.standard_normal((512, D)) * 0.02).astype(np.float32)
    Wp = (rng.standard_normal((D, D)) * 0.02).astype(np.float32)
    g = np.ones(H, np.float32)
    out = kernel(x, Wq, Wk, Wv, Wp, g)
    print("out", out.shape, out.dtype, float(np.abs(out).max()))


# revision 17
# speedup vs baseline: 1.4252x; 1.0867x over previous
"""Causal self-attention (GQA + RMS-norm + RoPE) Trainium2 Bass kernel.

Sharding over 8 NeuronCores: 2-way data parallel (batch) x 4-way head
parallel (one GQA group of 4 q-heads + 1 kv-head per core).  Each core
computes q/k/v projections for its group, flash-style causal attention
(scores kept transposed [k, q] so softmax sums ride the tensor engine),
and a partial output projection.  Host sums the 4 per-group partials per
batch.

Engine-balance notes (from NTFF traces):
 - All activations stay in one ACT table set (natural_log_exp_and_others):
   RMS-norm rsqrt and the softmax 1/d are exp(-ln(x)) instead of
   sqrt/reciprocal.  DVE's iterative-divide reciprocal costs ~3.3us per
   [1,512] row; ln+exp on ACT is ~1.4us and avoids table switching.
 - RoPE stages are bf16 so the DVE tensor-tensor ops run in 2x mode.
 - x loads are column-major (j-block at a time) so the first projection
   chains start ~12us into the kernel instead of after the full 8MB load.
 - Off-diagonal attention tiles run the PV and denominator matmuls in
   fp8 DoubleRow pairs (2 k-tiles per instruction); exp emits P in fp8
   with bias -5 (cancels in y/d).  Verified <=4.2e-3 rel err vs 2e-2 gate.
 - Output partials are bf16 (halves the store and the host gather).
"""

import numpy as np
import ml_dtypes

import concourse.bacc as bacc
import concourse.mybir as mybir
from concourse.tile import TileContext
from concourse.bass_utils import run_bass_kernel_spmd
from concourse.hw_specs import get_activation_tables as _get_act_tables

# Pin every activation to the one table set that covers exp/ln/square/copy.
# The insertion pass picks the first set containing each function, which
# otherwise thrashes between exp_and_others and natural_log (one ~2.7us
# ACT_TABLE_LOAD per ln<->exp alternation, ~73 loads per kernel).  Emptying
# the other sets (order/IDs preserved) forces a single hoisted load.
_PIN_SET = "natural_log_exp_and_others"


def _pinned_act_tables(arch):
    tables = _get_act_tables(arch)
    return {name: (fns if name == _PIN_SET else set())
            for name, fns in tables.items()}


bacc.get_activation_tables = _pinned_act_tables

BF16 = mybir.dt.bfloat16
F32 = mybir.dt.float32
F32R = mybir.dt.float32r
FP8 = mybir.dt.float8e4
AF = mybir.ActivationFunctionType
DR = mybir.MatmulPerfMode.DoubleRow
bf = ml_dtypes.bfloat16
f8 = ml_dtypes.float8_e4m3

B, S, D = 2, 2048, 2048
H, HKV, HD = 16, 4, 128
RQ = H // HKV            # q heads per kv group (4)
NCORES = 8
NDT = D // 128           # 16 contraction tiles
NST = S // 512           # 4 query/sequence 512-tiles
NKT = S // 128           # 16 key 128-tiles
EPS = float(np.finfo(np.float32).eps)
EXPB = -5.0              # softmax exp bias: P = e^(s-5); cancels in y/d

_PROG_CACHE = {}


def _build_program(n_timing_iters=1, phases="full"):
    nc = bacc.Bacc("TRN2", debug=False, enable_asserts=False, num_devices=NCORES)

    xT_d = nc.dram_tensor("xT", [128, NDT, S], BF16, kind="ExternalInput")
    wqT_d = nc.dram_tensor("wqT", [128, NDT, RQ * HD], BF16, kind="ExternalInput")
    wkT_d = nc.dram_tensor("wkT", [128, NDT, HD], BF16, kind="ExternalInput")
    wvT_d = nc.dram_tensor("wvT", [128, NDT, HD], BF16, kind="ExternalInput")
    wpT_d = nc.dram_tensor("wpT", [NDT, 128, RQ * 128], BF16, kind="ExternalInput")
    cosF_d = nc.dram_tensor("cosF", [128, S], BF16, kind="ExternalInput")
    sinF_d = nc.dram_tensor("sinF", [128, S], BF16, kind="ExternalInput")
    cfs_d = nc.dram_tensor("cfs", [1, 641], F32R, kind="ExternalInput")
    onescol_f_d = nc.dram_tensor("onescol_f", [128, 1], F32R, kind="ExternalInput")
    expb_d = nc.dram_tensor("expb", [128, 1], F32R, kind="ExternalInput")
    onescol_b_d = nc.dram_tensor("onescol_b", [128, 1], BF16, kind="ExternalInput")
    ones8_d = nc.dram_tensor("ones8", [128, 32], FP8, kind="ExternalInput")
    idtr_d = nc.dram_tensor("idtr", [128, 256], BF16, kind="ExternalInput")
    outT_d = nc.dram_tensor("outT", [NDT, 128, S], BF16, kind="ExternalOutput")

    with TileContext(nc) as tc:
        with tc.tile_pool(name="res", bufs=1) as res, \
             tc.tile_pool(name="work", bufs=2) as wk, \
             tc.tile_pool(name="pwork", bufs=2, space="PSUM") as pw:

            # ---- resident tiles (allocated once) ----
            xT = res.tile([128, NDT, S], BF16)             # [d-part, dt, s]
            wqT = res.tile([128, NDT, RQ * HD], BF16)
            wkT = res.tile([128, NDT, HD], BF16)
            wvT = res.tile([128, NDT, HD], BF16)
            cosF = res.tile([128, S], BF16)
            sinF = res.tile([128, S], BF16)
            cfs = res.tile([1, 641], F32R)
            onescol_f = res.tile([128, 1], F32R)
            expb = res.tile([128, 1], F32R)
            onescol_b = res.tile([128, 1], BF16)
            ones8 = res.tile([128, 2, 16], FP8)
            idtr = res.tile([128, 256], BF16)
            kT = res.tile([128, S], BF16)
            qT = [res.tile([128, S], BF16, name=f"qT{h}", tag=f"qT{h}")
                  for h in range(RQ)]
            yT = [res.tile([128, S], BF16, name=f"yT{h}", tag=f"yT{h}")
                  for h in range(RQ)]
            vTst = res.tile([128, S], BF16)                # v^T staging
            V_all = res.tile([128, S], BF16)               # v natural, kt-major
            V8 = res.tile([128, NKT, 128], FP8)            # fp8 copy of V_all

            eps_ap = cfs[0:1, 0:1].bitcast(F32)
            expb_ap = expb[:, 0:1].bitcast(F32)
            ident = idtr[:, 0:128]
            triu = idtr[:, 128:256]

            def body(_iv=None):
                # ---- load residents ----
                # Ordered by first use: k weights + x block 0 gate the first
                # chain (~8us), v weights + later x blocks stream behind,
                # cos/sin only gate the (latency-tolerant) rope tails, q
                # weights are needed ~30us in, attention consts later still.
                nc.sync.dma_start(wkT[:], wkT_d[:])
                nc.sync.dma_start(cfs[:], cfs_d[:])
                nc.sync.dma_start(onescol_f[:], onescol_f_d[:])

                def xblk(j):
                    sl = slice(512 * j, 512 * j + 512)
                    nc.sync.dma_start(xT[:, 0:8, sl], xT_d[:, 0:8, sl])
                    nc.sync.dma_start(xT[:, 8:16, sl], xT_d[:, 8:16, sl])

                xblk(0)
                nc.sync.dma_start(wvT[:], wvT_d[:])
                xblk(1)
                nc.sync.dma_start(cosF[:], cosF_d[:])
                nc.sync.dma_start(sinF[:], sinF_d[:])
                xblk(2)
                xblk(3)
                nc.sync.dma_start(wqT[:, 0:8, :], wqT_d[:, 0:8, :])
                nc.sync.dma_start(wqT[:, 8:16, :], wqT_d[:, 8:16, :])
                nc.sync.dma_start(expb[:], expb_d[:])
                nc.sync.dma_start(onescol_b[:], onescol_b_d[:])
                nc.sync.dma_start(ones8[:], ones8_d[:])
                nc.sync.dma_start(idtr[:], idtr_d[:])

                def proj_accum(wt_all, col_off, j, tag="big", tbufs=3):
                    """psum [128,512] = sum_d W[d].T @ xT[d, s-slice]"""
                    acc = pw.tile([128, 512], F32, name="acc", tag=tag,
                                  bufs=tbufs)
                    for dt in range(NDT):
                        nc.tensor.matmul(acc[:],
                                         wt_all[:, dt, col_off:col_off + 128],
                                         xT[:, dt, 512 * j:512 * j + 512],
                                         start=(dt == 0), stop=(dt == NDT - 1))
                    return acc

                def norm_rope_chain(acc, lg_ap, dest, j):
                    """RMS-norm + RoPE + scale; writes dest[:, 512j:+512] bf16.

                    rsqrt (and the q gain) ride ACT as exp(-0.5*ln(ms)+lg) so
                    everything stays in one activation table set.  Stages are
                    bf16 so the DVE rope ops run in 2x mode."""
                    stage = wk.tile([128, 512], BF16, name="stage", tag="stage", bufs=3)
                    nc.vector.tensor_copy(stage[:], acc[:])
                    swap = wk.tile([128, 512], BF16, name="swap", tag="swap", bufs=3)
                    nc.sync.dma_start(swap[0:64, :], stage[64:128, :])
                    nc.sync.dma_start(swap[64:128, :], stage[0:64, :])
                    sq = wk.tile([128, 512], BF16, name="sq", tag="sq", bufs=2)
                    nc.vector.tensor_mul(sq[:], stage[:], stage[:])
                    ms = pw.tile([1, 512], F32, name="ms", tag="bcast", bufs=1)
                    nc.tensor.matmul(ms[:], onescol_b[:],
                                     sq[:], start=True, stop=True)
                    lnms = wk.tile([1, 512], F32, name="lnms", tag="lnms", bufs=2)
                    nc.scalar.activation(lnms[:], ms[:], AF.Ln,
                                         bias=eps_ap, scale=1.0 / HD)
                    rg = wk.tile([1, 512], BF16, name="rg", tag="rg", bufs=2)
                    nc.scalar.activation(rg[:], lnms[:], AF.Exp,
                                         bias=(lg_ap if lg_ap is not None
                                               else 0.0), scale=-0.5)
                    Rb = wk.tile([128, 512], BF16, name="Rb", tag="Rb", bufs=2)
                    nc.gpsimd.partition_broadcast(Rb[:], rg[0:1, :])
                    sl = slice(512 * j, 512 * j + 512)
                    nc.vector.tensor_mul(stage[:], stage[:], cosF[:, sl])
                    nc.vector.tensor_mul(swap[:], swap[:], sinF[:, sl])
                    nc.vector.tensor_add(stage[:], stage[:], swap[:])
                    nc.vector.tensor_mul(dest[:, sl], stage[:], Rb[:])

                # ---- projections: mm-chains pipelined one ahead of stats ----
                proj_jobs = []   # (kind, h, j) interleaved to match x arrival
                for j in range(NST):
                    proj_jobs.append(("k", 0, j))
                    proj_jobs.append(("v", 0, j))

                def emit_proj_tail(kind, h, j, acc):
                    if kind == "k":
                        norm_rope_chain(acc, None, kT, j)
                    elif kind == "q":
                        lg_h = cfs[0:1, 133 + h:134 + h].bitcast(F32)
                        norm_rope_chain(acc, lg_h, qT[h], j)
                    else:
                        nc.scalar.copy(vTst[:, 512 * j:512 * j + 512], acc[:])

                def run_proj_jobs(jobs, pending, tags=None, lag=1):
                    for idx, (kind, h, j) in enumerate(jobs):
                        tag, tbufs = ("big", 3) if tags is None else tags[idx]
                        if kind == "k":
                            acc = proj_accum(wkT, 0, j, tag, tbufs)
                        elif kind == "v":
                            acc = proj_accum(wvT, 0, j, tag, tbufs)
                        else:
                            acc = proj_accum(wqT, 128 * h, j, tag, tbufs)
                        pending.append((kind, h, j, acc))
                        if len(pending) > lag:
                            emit_proj_tail(*pending.pop(0))
                    return pending

                # first wave: 7-deep psum concurrency so the x-load DMA
                # wavefront keeps PE fed (every arriving x tile unlocks mms)
                kv_tags = [("big", 3), ("big", 3), ("big", 3), ("acc", 2),
                           ("acc", 2), ("small", 2), ("small", 2), ("big", 3)]
                pending = run_proj_jobs(proj_jobs, [], tags=kv_tags, lag=6)

                # ---- v^T -> V transposes (PE); V kept in bf16 and fp8 ----
                while pending:
                    emit_proj_tail(*pending.pop(0))
                for kt in range(NKT):
                    tp = pw.tile([128, 128], BF16, name="tp", tag="acc", bufs=2)
                    nc.tensor.transpose(tp[:], vTst[:, 128 * kt:128 * kt + 128],
                                        ident)
                    nc.scalar.copy(V_all[:, 128 * kt:128 * kt + 128], tp[:])
                    nc.vector.tensor_copy(V8[:, kt, :], tp[:])

                if phases == "kv":
                    return

                def attention_block(h, j):
                    """Causal attention for queries [512j, 512j+512), head h.

                    Off-diagonal k-tiles (kt < 4j) are consumed as fp8
                    DoubleRow pairs (P in fp8 from exp, V from V8): one PV and
                    one denominator matmul per TWO k-tiles.  Diagonal tiles
                    stay bf16 with column trimming + triu mask.  Consumers
                    trail the S-matmul/exp pipeline so PE never waits on ACT."""
                    nkt = 4 * j + 4
                    npair = (4 * j) // 2
                    acc_y = pw.tile([128, 512], F32, name="acc_y", tag="acc",
                                    bufs=2)
                    acc_d = pw.tile([1, 512], F32, name="acc_d", tag="small",
                                    bufs=2)
                    ncons = npair + 4
                    lagged = []

                    def consume(ci, kind, P, c0):
                        first, last = ci == 0, ci == ncons - 1
                        if kind == "pair":
                            nc.tensor.matmul(acc_d[:], ones8[:, 0:2, 0:1],
                                             P[:, 0:2, :],
                                             start=first, stop=last,
                                             perf_mode=DR,
                                             skip_group_check=True)
                            nc.tensor.matmul(acc_y[:], V8[:, c0:c0 + 2, :],
                                             P[:, 0:2, :],
                                             start=first, stop=last,
                                             perf_mode=DR,
                                             skip_group_check=True)
                        else:
                            kt = c0
                            cc = 128 * (kt - 4 * j)
                            nc.tensor.matmul(acc_d[0:1, cc:512], onescol_b[:],
                                             P[:, cc:512],
                                             start=first, stop=last,
                                             skip_group_check=True)
                            nc.tensor.matmul(acc_y[:, cc:512],
                                             V_all[:, 128 * kt:128 * kt + 128],
                                             P[:, cc:512],
                                             start=first, stop=last,
                                             skip_group_check=True)

                    # interleave pair and diag units: the pair phase is
                    # ACT-bound (two 512-wide exps per ~0.9us of PE) while the
                    # diag phase has ACT slack, so alternating keeps both fed.
                    units = []
                    pu, dk = 0, 4 * j
                    while pu < npair or dk < nkt:
                        if pu < npair:
                            units.append(("pair", pu))
                            pu += 1
                        if dk < nkt:
                            units.append(("diag", dk))
                            dk += 1
                    ci = 0
                    for kind, idx in units:
                        if kind == "pair":
                            P8 = wk.tile([128, 2, 512], FP8, name="P8",
                                         tag="P8", bufs=4)
                            for u in (0, 1):
                                kt = 2 * idx + u
                                ps = pw.tile([128, 512], F32, name="ps",
                                             tag="big", bufs=3)
                                nc.tensor.matmul(
                                    ps[:], kT[:, 128 * kt:128 * kt + 128],
                                    qT[h][:, 512 * j:512 * j + 512],
                                    start=True, stop=True)
                                nc.scalar.activation(P8[:, u, :], ps[:],
                                                     AF.Exp, bias=expb_ap)
                            lagged.append((ci, "pair", P8, 2 * idx))
                        else:
                            kt = idx
                            cc = 128 * (kt - 4 * j)
                            ps = pw.tile([128, 512], F32, name="ps", tag="big",
                                         bufs=3)
                            nc.tensor.matmul(
                                ps[:, cc:512],
                                kT[:, 128 * kt:128 * kt + 128],
                                qT[h][:, 512 * j + cc:512 * j + 512],
                                start=True, stop=True)
                            P = wk.tile([128, 512], BF16, name="P", tag="P",
                                        bufs=4)
                            nc.scalar.activation(P[:, cc:512], ps[:, cc:512],
                                                 AF.Exp, bias=expb_ap)
                            nc.vector.tensor_mul(P[:, cc:cc + 128],
                                                 P[:, cc:cc + 128], triu)
                            lagged.append((ci, "diag", P, kt))
                        ci += 1
                        if len(lagged) > 2:
                            consume(*lagged.pop(0))
                    while lagged:
                        consume(*lagged.pop(0))
                    # 1/d on DVE (custom Newton-seed op, ~18-bit accurate);
                    # keeps ACT free for the exp stream it bounds.
                    rdf = wk.tile([1, 512], F32, name="rdf", tag="lnms", bufs=2)
                    nc.vector.reciprocal_approx_fast(rdf[:], acc_d[:])
                    Rd = wk.tile([128, 512], F32, name="Rd", tag="Rdf", bufs=2)
                    nc.gpsimd.partition_broadcast(Rd[:], rdf[0:1, :])
                    nc.vector.tensor_mul(yT[h][:, 512 * j:512 * j + 512],
                                         acc_y[:], Rd[:])

                # ---- per q-head: q(h+1) projections emitted ahead of
                # attention(h) so PE crosses head boundaries without gaps ----
                pending = run_proj_jobs([("q", 0, j) for j in range(NST)],
                                        pending)
                for h in range(RQ):
                    if h + 1 < RQ:
                        pending = run_proj_jobs(
                            [("q", h + 1, j) for j in range(NST)], pending)
                    while pending:
                        emit_proj_tail(*pending.pop(0))
                    if phases == "kvq":
                        continue
                    for j in range(NST):
                        attention_block(h, j)

                # ---- output projection (transposed: out^T[D, s]) ----
                if phases in ("kv", "kvq", "noout"):
                    return
                ptags = ["big", "acc", "bcast", "small"]
                pbufs = {"big": 3, "acc": 2, "bcast": 1, "small": 2}
                for dt in range(NDT):
                    wp = wk.tile([128, RQ * 128], BF16, name="wp", tag="wp",
                                 bufs=3)
                    nc.sync.dma_start(wp[:], wpT_d[dt])
                    osb = wk.tile([128, S], BF16, name="osb", tag="osb")
                    for sjj in range(NST):
                        po = pw.tile([128, 512], F32, name=f"po{sjj}",
                                     tag=ptags[sjj], bufs=pbufs[ptags[sjj]])
                        for h in range(RQ):
                            nc.tensor.matmul(
                                po[:], wp[:, 128 * h:128 * h + 128],
                                yT[h][:, 512 * sjj:512 * sjj + 512],
                                start=(h == 0), stop=(h == RQ - 1))
                        # psum->sbuf copies split across ACT and DVE
                        if sjj % 2 == 0:
                            nc.scalar.copy(osb[:, 512 * sjj:512 * sjj + 512],
                                           po[:])
                        else:
                            nc.vector.tensor_copy(
                                osb[:, 512 * sjj:512 * sjj + 512], po[:])
                    nc.sync.dma_start(outT_d[dt], osb[:])

            if n_timing_iters > 1:
                with tc.For_i(0, n_timing_iters, 1):
                    body()
            else:
                body()

    nc.compile()
    return nc


def _get_program(n_timing_iters=1, phases="full"):
    key = (n_timing_iters, phases)
    if key not in _PROG_CACHE:
        _PROG_CACHE[key] = _build_program(n_timing_iters, phases)
    return _PROG_CACHE[key]


def _host_inputs(x, Wq, Wk, Wv, Wproj, q_gain):
    """Build the 8 per-core input maps (host-side layout prep)."""
    inv = 1.0 / (10000.0 ** (np.arange(0, HD, 2, dtype=np.float64) / HD))
    t = np.arange(S, dtype=np.float64)
    fr = np.outer(t, inv).astype(np.float32)          # [S, 64]
    cos = np.cos(fr).astype(np.float32)
    sin = np.sin(fr).astype(np.float32)
    cosF = np.concatenate([cos.T, cos.T], 0).astype(bf)          # [128, S]
    sinF = np.concatenate([sin.T, -sin.T], 0).astype(bf)

    onescol_f = np.ones((128, 1), np.float32)
    onescol_b = np.ones((128, 1), bf)
    ones8 = np.ones((128, 32), f8)
    ident = np.eye(128, dtype=np.float32)
    triu = (np.arange(128)[None, :] >= np.arange(128)[:, None]).astype(np.float32)
    idtr = np.concatenate([ident, triu], 1).astype(bf)

    # fold gain sign into Wq rows; |gain| rides the rsqrt exp as a ln-bias
    gsign = np.where(q_gain < 0, -1.0, 1.0).astype(np.float32)
    Wq = Wq * np.repeat(gsign, HD)[:, None]
    glog = np.log(np.maximum(np.abs(q_gain.astype(np.float64)), 1e-300)
                  / np.sqrt(HD))
    glog = np.maximum(glog, -80.0).astype(np.float32)

    # [128, NDT, S]: xT[p, dt, s] = x[b][s, 128*dt+p]
    xT = [np.ascontiguousarray(
        x[b].T.reshape(NDT, 128, S).transpose(1, 0, 2)).astype(bf)
        for b in range(B)]

    in_maps = []
    for c in range(NCORES):
        b, g = c // HKV, c % HKV
        wq = np.ascontiguousarray(Wq[512 * g:512 * (g + 1)].T)   # [D, 512]
        wk_ = np.ascontiguousarray(Wk[128 * g:128 * (g + 1)].T)  # [D, 128]
        wv = np.ascontiguousarray(Wv[128 * g:128 * (g + 1)].T)
        wpT = np.ascontiguousarray(Wproj[:, 512 * g:512 * (g + 1)].T)  # [512, 2048]
        # [dt][c-part 128, (h, m) 512]: wpT2[dt, c, 128h+m] = Wp[128dt+m, 512g+128h+c]
        wpT = np.ascontiguousarray(
            wpT.reshape(RQ, 128, NDT, 128).transpose(2, 1, 0, 3).reshape(
                NDT, 128, RQ * 128)).astype(bf)
        expb_col = np.full((128, 1), EXPB, np.float32)
        cfsv = np.zeros((1, 641), np.float32)
        cfsv[0, 0] = EPS
        cfsv[0, 1:129] = 1.0
        cfsv[0, 129:133] = (np.abs(q_gain[RQ * g: RQ * (g + 1)])
                            / np.sqrt(HD)).astype(np.float32)
        cfsv[0, 133:137] = glog[RQ * g: RQ * (g + 1)]
        in_maps.append({
            "xT": xT[b],
            "wqT": np.ascontiguousarray(
                wq.reshape(NDT, 128, RQ * HD).transpose(1, 0, 2)).astype(bf),
            "wkT": np.ascontiguousarray(
                wk_.reshape(NDT, 128, HD).transpose(1, 0, 2)).astype(bf),
            "wvT": np.ascontiguousarray(
                wv.reshape(NDT, 128, HD).transpose(1, 0, 2)).astype(bf),
            "wpT": wpT,
            "cosF": cosF, "sinF": sinF, "cfs": cfsv,
            "onescol_f": onescol_f, "onescol_b": onescol_b,
            "ones8": ones8, "idtr": idtr, "expb": expb_col,
        })
    return in_maps


def kernel(x, Wq, Wk, Wv, Wproj, q_gain, _n_timing_iters=1, _return_raw=False,
           _trace=False):
    x = np.asarray(x, np.float32)
    in_maps = _host_inputs(np.asarray(x, np.float32),
                           np.asarray(Wq, np.float32),
                           np.asarray(Wk, np.float32),
                           np.asarray(Wv, np.float32),
                           np.asarray(Wproj, np.float32),
                           np.asarray(q_gain, np.float32))
    nc = _get_program(_n_timing_iters)
    res = run_bass_kernel_spmd(nc, in_maps, core_ids=list(range(NCORES)),
                               trace=_trace)
    if _return_raw:
        return res
    out = np.zeros((B, S, D), np.float32)
    for c in range(NCORES):
        b = c // HKV
        outT = res.results[c]["outT"].astype(np.float32).reshape(D, S)
        out[b] += outT.T
    return out


if __name__ == "__main__":
    rng = np.random.default_rng(0)
    x = rng.standard_normal((B, S, D)).astype(np.float32)
    Wq = (rng.standard_normal((D, D)) * 0.02).astype(np.float32)
    Wk = (rng.standard_normal((512, D)) * 0.02).astype(np.float32)
    Wv = (rng.standard_normal((512, D)) * 0.02).astype(np.float32)
    Wp = (rng.standard_normal((D, D)) * 0.02).astype(np.float32)
    g = np.ones(H, np.float32)
    out = kernel(x, Wq, Wk, Wv, Wp, g)
    print("out", out.shape, out.dtype, float(np.abs(out).max()))
